# revision 38
# baseline (speedup 1.0000x reference)
"""BIMPM forward kernel for 8 Trainium2 NeuronCores (data-parallel over batch).

Shapes (hardcoded): B=32, S1=S2=60, H=100, L=20, D=300, V=30000, CLS=3.
Each core handles B_local=4 batch rows end-to-end:
  embed gather -> context BiLSTM -> multi-perspective matching -> agg BiLSTM -> FC.
"""

import numpy as np

import concourse.bacc as bacc
import concourse.bass as bass
import concourse.mybir as mybir
import concourse.tile as tile
from concourse.bass import AP, IndirectOffsetOnAxis
from concourse.bass_utils import run_bass_kernel_spmd

F32 = mybir.dt.float32
I32 = mybir.dt.int32
AF = mybir.ActivationFunctionType
OP = mybir.AluOpType
AX = mybir.AxisListType

B, S, H, L, D, V, CLS = 32, 60, 100, 20, 300, 30000, 3
NCORE = 8
BL = B // NCORE          # 4 batch rows per core
NSEQ = 2 * BL            # 8 sequences (p rows then h rows) per scan chain
NTOK = S * NSEQ          # 480 (t,s) token columns
DH = 4 * H               # 400
AGG_IN = 8 * L           # 160
# gate order used on-chip: [i, f, o, g]; torch rows are [i, f, g, o]
GSRC = [0, 1, 3, 2]
GBANK = 512              # psum column pitch per gate block (bank aligned)

PARAM_NAMES = [
    "word_emb",
    "ctx_Wih_f", "ctx_Whh_f", "ctx_bih_f", "ctx_bhh_f",
    "ctx_Wih_b", "ctx_Whh_b", "ctx_bih_b", "ctx_bhh_b",
    "w1", "w2", "w3", "w4", "w5", "w6", "w7", "w8",
    "agg_Wih_f", "agg_Whh_f", "agg_bih_f", "agg_bhh_f",
    "agg_Wih_b", "agg_Whh_b", "agg_bih_b", "agg_bhh_b",
    "fc1_W", "fc1_b", "fc2_W", "fc2_b",
]


def apv(base, poff, foff, fdims, pcnt=None):
    """Build a strided view of a 2D tile AP.

    base: AP of a (P, F) tile; poff/pcnt: partition offset/count;
    foff: free offset (elements); fdims: list of (step, count) free dims.
    """
    row = base.ap[0][0]
    pc = base.ap[0][1] if pcnt is None else pcnt
    ap = [[row, pc]] + [[s, c] for (s, c) in fdims]
    return AP(base.tensor, base.offset + poff * row + foff, ap)


def revt(base, poff, pcnt, foff, fdims):
    """Like apv but with the FIRST free dim reversed (negative step)."""
    row = base.ap[0][0]
    (s0, c0) = fdims[0]
    off = base.offset + poff * row + foff + s0 * (c0 - 1)
    ap = [[row, pcnt]] + [[-s0, c0]] + [[s, c] for (s, c) in fdims[1:]]
    return AP(base.tensor, off, ap)


def emit(nc, tc, io):
    from contextlib import ExitStack
    es = ExitStack()
    v = nc.vector
    sc = nc.scalar
    te = nc.tensor
    gp = nc.gpsimd
    sy = nc.sync

    CP = es.enter_context(tc.tile_pool(name="const", bufs=1))
    WK = es.enter_context(tc.tile_pool(name="work", bufs=2))

    # ---------------- identities / small constants ----------------
    ident = CP.tile([128, 128], F32, name="ident")
    from concourse.masks import make_identity
    make_identity(nc, ident[:])

    onesH = CP.tile([H, 1], F32, name="onesH")
    gp.memset(onesH[:], 1.0)
    ones60 = CP.tile([1, 60], F32, name="ones60")
    gp.memset(ones60[:], 1.0)

    ones_row = CP.tile([1, NTOK], F32, name="ones_row")
    gp.memset(ones_row[:], 1.0)

    # ---------------- w1..w8 -> w_all / wsq_all ----------------
    # column layout: [w1 w5 w7 w3 | w2 w6 w8 w4] (fwd singles, fwd max | bwd ...)
    w_order = ["w1", "w5", "w7", "w3", "w2", "w6", "w8", "w4"]
    w_all = CP.tile([H, 160], F32, name="w_all")
    for i, wn in enumerate(w_order):
        sy.dma_start(out=w_all[:, i * L:(i + 1) * L], in_=io[wn][:])
    wsq = CP.tile([H, 160], F32, name="wsq")
    v.tensor_tensor(out=wsq[:], in0=w_all[:], in1=w_all[:], op=OP.mult)

    # ---------------- embedding gather + transpose to xT ----------------
    # index tiles: 2 chunks of 120 per side (b-pair major, row-major (b,t))
    from contextlib import ExitStack as _ES
    pp_es = _ES()
    PP = pp_es.enter_context(tc.tile_pool(name="prep_ps", bufs=2, space="PSUM"))

    xT = [CP.tile([100, NTOK], F32, name=f"xT{k}") for k in range(2)]
    xT.append(CP.tile([101, NTOK], F32, name="xT2"))
    sy.dma_start(out=xT[2][100:101, :], in_=ones_row[:])  # bias-ones row

    for side, pname in ((0, "p"), (1, "h")):
        idx_flat = io[pname].rearrange("a (b u) -> (a b) u", u=1)
        for ch in range(2):
            itile = WK.tile([120, 1], I32, tag="idx", name=f"idx_{side}_{ch}")
            sy.dma_start(out=itile[:], in_=idx_flat[ch * 120:(ch + 1) * 120])
            emb = WK.tile([120, D], F32, tag="emb", name=f"emb_{side}_{ch}")
            gp.indirect_dma_start(
                out=emb[:], out_offset=None, in_=io["word_emb"][:],
                in_offset=IndirectOffsetOnAxis(ap=itile[:, :1], axis=0),
            )
            for hc in range(3):
                ps = PP.tile([100, 120], F32, tag="tp", name=f"tp_{side}_{ch}_{hc}")
                te.transpose(out=ps[:], in_=emb[:, hc * 100:(hc + 1) * 100],
                             identity=ident[:120, :120])
                # psum cols n = b2*60+t ; xT col = t*8 + (side*4 + ch*2 + b2)
                dst = apv(xT[hc][:], 0, side * 4 + ch * 2, [(1, 2), (8, 60)], pcnt=100)
                src = apv(ps[:], 0, 0, [(60, 2), (1, 60)])
                v.tensor_copy(out=dst, in_=src)

    # reversed-time copies for the bwd chain input
    xTr = [CP.tile([100, NTOK], F32, name=f"xTr{k}") for k in range(2)]
    xTr.append(CP.tile([101, NTOK], F32, name="xTr2"))
    for k in range(3):
        pc = 101 if k == 2 else 100
        v.tensor_copy(out=apv(xTr[k][:], 0, 0, [(8, 60), (1, 8)], pcnt=pc),
                      in_=revt(xT[k][:], 0, pc, 0, [(8, 60), (1, 8)]))

    # ---------------- LSTM weight prep ----------------
    def prep_lstm(pre, din, nchunk, chunks):
        """Returns (WihT list per k-chunk, WhhT per ws, last chunk has bias row)."""
        wihT = {}
        whhT = {}
        for ws in ("f", "b"):
            wt = []
            for g in range(4):
                t0 = WK.tile([100, din], F32, tag="wld", bufs=4, name=f"wld_{pre}_{ws}_{g}")
                sy.dma_start(out=t0[:], in_=io[f"{pre}_Wih_{ws}"][GSRC[g] * H:(GSRC[g] + 1) * H, :])
                wt.append(t0)
            ks = []
            off = 0
            for k, csz in enumerate(chunks):
                pc = csz + (1 if k == nchunk - 1 else 0)
                kt = CP.tile([pc, DH], F32, name=f"wihT_{pre}_{ws}_{k}")
                ps = PP.tile([csz, DH], F32, tag="wtp", name=f"wtp_{pre}_{ws}_{k}")
                for g in range(4):
                    te.transpose(out=ps[:, g * H:(g + 1) * H],
                                 in_=wt[g][:, off:off + csz],
                                 identity=ident[:100, :100])
                v.tensor_copy(out=kt[0:csz, 0:3 * H], in_=ps[:, 0:3 * H])
                v.tensor_scalar_mul(out=kt[0:csz, 3 * H:DH], in0=ps[:, 3 * H:DH], scalar1=2.0)
                ks.append(kt)
                off += csz
            wihT[ws] = ks

            # bias row: bih + bhh, gate-reordered, g-gate doubled
            b1 = WK.tile([1, DH], F32, tag="brow", bufs=4, name=f"b1_{pre}_{ws}")
            b2 = WK.tile([1, DH], F32, tag="brow", bufs=4, name=f"b2_{pre}_{ws}")
            sy.dma_start(out=b1[:], in_=io[f"{pre}_bih_{ws}"].rearrange("(u a) -> u a", u=1))
            sy.dma_start(out=b2[:], in_=io[f"{pre}_bhh_{ws}"].rearrange("(u a) -> u a", u=1))
            badd = WK.tile([1, DH], F32, tag="brow", bufs=4, name=f"badd_{pre}_{ws}")
            v.tensor_tensor(out=badd[:], in0=b1[:], in1=b2[:], op=OP.add)
            brow = WK.tile([1, DH], F32, tag="brow", bufs=4, name=f"brow_{pre}_{ws}")
            for g in range(4):
                src = badd[:, GSRC[g] * H:(GSRC[g] + 1) * H]
                if g == 3:
                    v.tensor_scalar_mul(out=brow[:, g * H:(g + 1) * H],
                                        in0=src, scalar1=2.0)
                else:
                    v.tensor_copy(out=brow[:, g * H:(g + 1) * H], in_=src)
            last = ks[-1]
            lrow = chunks[-1]
            sy.dma_start(out=last[lrow:lrow + 1, :], in_=brow[:])

            # Whh^T  (100, 400), g-gate doubled
            wh = []
            for g in range(4):
                t0 = WK.tile([100, H], F32, tag="wld2", bufs=4, name=f"whld_{pre}_{ws}_{g}")
                sy.dma_start(out=t0[:], in_=io[f"{pre}_Whh_{ws}"][GSRC[g] * H:(GSRC[g] + 1) * H, :])
                wh.append(t0)
            ps = PP.tile([H, DH], F32, tag="wtp", name=f"whtp_{pre}_{ws}")
            for g in range(4):
                te.transpose(out=ps[:, g * H:(g + 1) * H], in_=wh[g][:],
                             identity=ident[:100, :100])
            ht = CP.tile([H, DH], F32, name=f"whhT_{pre}_{ws}")
            v.tensor_copy(out=ht[:, 0:3 * H], in_=ps[:, 0:3 * H])
            v.tensor_scalar_mul(out=ht[:, 3 * H:DH], in0=ps[:, 3 * H:DH], scalar1=2.0)
            whhT[ws] = ht
        return wihT, whhT

    ctx_wihT, ctx_whhT = prep_lstm("ctx", D, 3, [100, 100, 100])
    agg_wihT, agg_whhT = prep_lstm("agg", AGG_IN, 2, [80, 80])

    # ---------------- FC weight prep ----------------
    fc1T = []
    fc1w = [WK.tile([100, DH], F32, tag="fcw", name=f"fc1w_{m}") for m in range(2)]
    for m in range(2):
        sy.dma_start(out=fc1w[m][:], in_=io["fc1_W"][m * 100:(m + 1) * 100, :])
    for k in range(4):
        ps = PP.tile([100, 200], F32, tag="wtp", name=f"fc1tp_{k}")
        for m in range(2):
            te.transpose(out=ps[:, m * 100:(m + 1) * 100],
                         in_=fc1w[m][:, k * 100:(k + 1) * 100],
                         identity=ident[:100, :100])
        kt = CP.tile([100, 200], F32, name=f"fc1T_{k}")
        v.tensor_copy(out=kt[:], in_=ps[:])
        fc1T.append(kt)
    fc2w = WK.tile([CLS, 200], F32, tag="fcw", name="fc2w")
    sy.dma_start(out=fc2w[:], in_=io["fc2_W"][:])
    fc2T = []
    for k in range(2):
        ps = PP.tile([100, CLS], F32, tag="wtp", name=f"fc2tp_{k}")
        te.transpose(out=ps[:], in_=fc2w[:, k * 100:(k + 1) * 100],
                     identity=ident[:CLS, :CLS])
        kt = CP.tile([100, CLS], F32, name=f"fc2T_{k}")
        v.tensor_copy(out=kt[:], in_=ps[:])
        fc2T.append(kt)
    fc1b = CP.tile([100, 2], F32, name="fc1b")
    sy.dma_start(out=fc1b[:], in_=io["fc1_b"].rearrange("(m p) -> p m", p=100))
    fc2b = CP.tile([CLS, 1], F32, name="fc2b")
    sy.dma_start(out=fc2b[:], in_=io["fc2_b"].rearrange("(a u) -> a u", u=1))

    pp_es.close()

    # ================= scan helper =================
    def run_scan(tag, wihT, whhT, rhs_k, chunks, seq):
        """One BiLSTM direction pair phase. rhs_k[ws] = list of xT-like tiles.
        seq[ws]: (100, 480) output tile (h_t at cols t*8+s)."""
        sps_es = _ES()
        SPS = sps_es.enter_context(tc.tile_pool(name=f"ps_{tag}", bufs=1, space="PSUM"))
        psx = {ws: SPS.tile([H, 4 * GBANK], F32, name=f"psx_{tag}_{ws}")
               for ws in ("f", "b")}
        # xp matmuls: psum[:, g*512 + t*8 + s] += WihT_k.T @ xT_k
        for ws in ("f", "b"):
            nk = len(chunks)
            for g in range(4):
                for k in range(nk):
                    pc = chunks[k] + (1 if k == nk - 1 else 0)
                    te.matmul(out=psx[ws][:, g * GBANK:g * GBANK + NTOK],
                              lhsT=wihT[ws][k][0:pc, g * H:(g + 1) * H],
                              rhs=rhs_k[ws][k][0:pc, :],
                              start=(k == 0), stop=(k == nk - 1))
        # scan
        ct = {}
        for ws in ("f", "b"):
            ct[ws] = CP.tile([H, 16], F32, name=f"ct_{tag}_{ws}")
            gp.memset(ct[ws][:, 8:16], 0.0)
        for t in range(S):
            for ws in ("f", "b"):
                ps, se, c = psx[ws], seq[ws], ct[ws]
                if t > 0:
                    for g in range(4):
                        te.matmul(out=ps[:, g * GBANK + t * 8: g * GBANK + t * 8 + 8],
                                  lhsT=whhT[ws][:, g * H:(g + 1) * H],
                                  rhs=se[:, (t - 1) * 8: t * 8],
                                  start=False, stop=True, skip_group_check=True)
                sig = WK.tile([H, 32], F32, tag=f"sig{ws}", bufs=3, name=f"sig_{tag}_{ws}_{t}")
                sc.activation(out=sig[:], in_=apv(ps[:], 0, t * 8, [(GBANK, 4), (1, 8)]),
                              func=AF.Sigmoid)
                # t1 = 2*sig_g - 1  -> ct[:,0:8]
                v.tensor_scalar(out=c[:, 0:8], in0=sig[:, 24:32],
                                scalar1=2.0, scalar2=1.0, op0=OP.mult, op1=OP.subtract)
                m = WK.tile([H, 16], F32, tag=f"m{ws}", name=f"m_{tag}_{ws}_{t}")
                v.tensor_tensor(out=m[:], in0=sig[:, 0:16], in1=c[:, 0:16], op=OP.mult)
                v.tensor_tensor(out=c[:, 8:16], in0=m[:, 0:8], in1=m[:, 8:16], op=OP.add)
                th = WK.tile([H, 8], F32, tag=f"th{ws}", name=f"th_{tag}_{ws}_{t}")
                sc.activation(out=th[:], in_=c[:, 8:16], func=AF.Tanh)
                v.tensor_tensor(out=se[:, t * 8:(t + 1) * 8], in0=sig[:, 16:24],
                                in1=th[:], op=OP.mult)
        sps_es.close()

    # ================= context scan =================
    seqc = {ws: CP.tile([H, NTOK], F32, name=f"seqc_{ws}") for ws in ("f", "b")}
    run_scan("ctx", ctx_wihT, ctx_whhT, {"f": xT, "b": xTr}, [100, 100, 100], seqc)

    # reversed copy of bwd seq -> original time order
    seqbr = CP.tile([H, NTOK], F32, name="seqbr")
    v.tensor_copy(out=apv(seqbr[:], 0, 0, [(8, 60), (1, 8)]),
                  in_=revt(seqc["b"][:], 0, 100, 0, [(8, 60), (1, 8)]))

    # ================= matching =================
    # mv feature layout per (b, side): (60 time, 160 feat);
    # feat block per dir(80): [full(20) max(20) am(20) amax(20)], dirs f then b.
    mvp = [CP.tile([S, AGG_IN], F32, name=f"mvp_{b}") for b in range(BL)]
    mvh = [CP.tile([S, AGG_IN], F32, name=f"mvh_{b}") for b in range(BL)]

    m_es = _ES()
    MP = m_es.enter_context(tc.tile_pool(name="match_ps", bufs=1, space="PSUM"))
    MPG = m_es.enter_context(tc.tile_pool(name="grp_ps", bufs=1, space="PSUM"))
    VWP = m_es.enter_context(tc.tile_pool(name="vwp", bufs=2))

    for di, dname in ((0, "f"), (1, "b")):
        seqd = seqc["f"] if di == 0 else seqbr
        sqd = CP.tile([H, NTOK], F32, name=f"sqd_{dname}")
        v.tensor_tensor(out=sqd[:], in0=seqd[:], in1=seqd[:], op=OP.mult)
        ut = S - 1 if di == 0 else 0     # "last_idx" column in orig time

        def vsl(base, s0, cnt=1):
            # (100, cnt*60) strided view: columns t*8+s for s in [s0, s0+cnt)
            return apv(base[:], 0, s0, [(1, cnt), (8, 60)]) if cnt > 1 else \
                   apv(base[:], 0, s0, [(8, 60)])

        # ---- position norms: rn_dir (1,480); bcast to 4 rows ----
        psn = MP.tile([1, NTOK], F32, tag="pA", bufs=2, name=f"psn_{dname}")
        te.matmul(out=psn[:], lhsT=onesH[:], rhs=sqd[:], start=True, stop=True)
        rc = WK.tile([1, NTOK], F32, tag="rc", name=f"rc_{dname}")
        v.reciprocal(out=rc[:], in_=psn[:])
        rn = CP.tile([1, NTOK], F32, name=f"rn_{dname}")
        sc.activation(out=rn[:], in_=rc[:], func=AF.Sqrt)

        # ---- (j,h) transposed copies of seq (per b), with ones column ----
        # rn columns (60,1) per sequence s, from rn row
        rncol = CP.tile([60, 8], F32, name=f"rncol_{dname}")
        for s in range(8):
            psn2 = MP.tile([60, 1], F32, tag="pA", bufs=2, name=f"rnc_{dname}_{s}")
            te.transpose(out=psn2[:], in_=apv(rn[:], 0, s, [(8, 60)], pcnt=1),
                         identity=ident[:1, :1])
            v.tensor_copy(out=rncol[:, s:s + 1], in_=psn2[:])
        jh = {}
        for side in (0, 1):
            tiles = []
            for b in range(BL):
                ps = MP.tile([60, 100], F32, tag="pA", bufs=2,
                             name=f"jhp_{dname}_{side}_{b}")
                te.transpose(out=ps[:], in_=vsl(seqd, side * 4 + b),
                             identity=ident[:100, :100])
                t0 = CP.tile([60, 101], F32, name=f"jh_{dname}_{side}_{b}")
                gp.memset(t0[:, 100:101], 1.0)
                v.tensor_copy(out=t0[:, 0:100], in_=ps[:])
                tiles.append(t0)
            jh[side] = tiles

        # ---- attention (am / amax u-vectors), target side ts gets u from other side ----
        umean = {}
        umax = {}
        for ts in (0, 1):                 # ts=0: target p (u from h side), ts=1: target h
            us = 1 - ts
            um = CP.tile([100, 4], F32, name=f"umean_{dname}_{ts}")
            ux = CP.tile([100, 4], F32, name=f"umax_{dname}_{ts}")
            for b in range(BL):
                # a0 column (60,1): cos(v1[b,0], v2[b,j]) modulo the |v1_0| factor
                ncol = MP.tile([60, 1], F32, tag="pA", bufs=2,
                               name=f"ncol_{dname}_{ts}_{b}")
                te.matmul(out=ncol[:], lhsT=vsl(seqd, us * 4 + b),
                          rhs=apv(seqd[:], 0, ts * 4 + b, [(1, 1)]),
                          start=True, stop=True)
                acol = WK.tile([60, 1], F32, tag="acol", name=f"acol_{dname}_{ts}_{b}")
                v.tensor_tensor(out=acol[:], in0=ncol[:],
                                in1=rncol[:, us * 4 + b:us * 4 + b + 1], op=OP.mult)
                # mean: (1,101) = acol.T @ [jh | 1]
                mr = MP.tile([1, 101], F32, tag="mr", bufs=1,
                             name=f"mraw_{dname}_{ts}_{b}")
                te.matmul(out=mr[:], lhsT=acol[:], rhs=jh[us][b][:, 0:101],
                          start=True, stop=True)
                rd = WK.tile([1, 1], F32, tag="rd", name=f"rd_{dname}_{ts}_{b}")
                v.tensor_scalar_add(out=rd[:], in0=mr[0:1, 100:101], scalar1=1e-10)
                v.reciprocal(out=rd[:], in_=rd[:])
                msc = WK.tile([1, 100], F32, tag="msc", name=f"msc_{dname}_{ts}_{b}")
                v.tensor_scalar(out=msc[:], in0=mr[0:1, 0:100],
                                scalar1=rd[0:1, 0:1], scalar2=None, op0=OP.mult)
                psu = MP.tile([100, 1], F32, tag="pA", bufs=2,
                              name=f"psu_{dname}_{ts}_{b}")
                te.transpose(out=psu[:], in_=msc[:], identity=ident[:1, :1])
                v.tensor_copy(out=um[:, b:b + 1], in_=psu[:])
                # max: scale jh rows by acol, transpose back, rowmax
                jsc = WK.tile([60, 100], F32, tag="jsc", name=f"jsc_{dname}_{ts}_{b}")
                v.tensor_scalar(out=jsc[:], in0=jh[us][b][:, 0:100],
                                scalar1=acol[:, 0:1], scalar2=None, op0=OP.mult)
                pst = MP.tile([100, 60], F32, tag="pB", bufs=1,
                              name=f"pst_{dname}_{ts}_{b}")
                te.transpose(out=pst[:], in_=jsc[:], identity=ident[:60, :60])
                v.reduce_max(out=ux[:, b:b + 1], in_=pst[:], axis=AX.X)
            umean[ts] = um
            umax[ts] = ux

        # ---- singles (full / am / amax) ----
        for ts in (0, 1):
            us = 1 - ts
            mvt = mvp if ts == 0 else mvh
            # U12 (100, 12): cols b*3 + {0:full,1:am,2:amax}
            U12 = WK.tile([100, 12], F32, tag="U12", name=f"U12_{dname}_{ts}")
            v.tensor_copy(out=apv(U12[:], 0, 0, [(3, 4)]),
                          in_=apv(seqd[:], 0, ut * 8 + us * 4, [(1, 4)]))
            v.tensor_copy(out=apv(U12[:], 0, 1, [(3, 4)]), in_=umean[ts][:])
            v.tensor_copy(out=apv(U12[:], 0, 2, [(3, 4)]), in_=umax[ts][:])
            U12s = WK.tile([100, 12], F32, tag="U12s", name=f"U12s_{dname}_{ts}")
            v.tensor_tensor(out=U12s[:], in0=U12[:], in1=U12[:], op=OP.mult)
            # u norms: 3 mm -> (4, 60) [cols wt*20+l]
            rups = MP.tile([1, 240], F32, tag="pA", bufs=2, name=f"rups_{dname}_{ts}")
            for b in range(BL):
                for wt in range(3):
                    te.matmul(out=rups[:, b * 60 + wt * L: b * 60 + (wt + 1) * L],
                              lhsT=U12s[:, b * 3 + wt: b * 3 + wt + 1],
                              rhs=wsq[:, di * 80 + wt * L: di * 80 + (wt + 1) * L],
                              start=True, stop=True)
            rur = WK.tile([1, 240], F32, tag="rur", name=f"rur_{dname}_{ts}")
            v.reciprocal(out=rur[:], in_=rups[:])
            RUf = WK.tile([1, 240], F32, tag="RUf", name=f"RUf_{dname}_{ts}")
            sc.activation(out=RUf[:], in_=rur[:], func=AF.Sqrt)
            rbA = MP.tile([60, 240], F32, tag="pB", bufs=1, name=f"rbA_{dname}_{ts}")
            te.matmul(out=rbA[:], lhsT=ones60[:], rhs=RUf[:], start=True, stop=True)
            # v1 norms for the 3 w's: (60, 240) [b*60 + wt*20 + l]
            sps = MP.tile([60, 240], F32, tag="pA", bufs=2, name=f"sps_{dname}_{ts}")
            for b in range(BL):
                te.matmul(out=sps[:, b * 60:(b + 1) * 60],
                          lhsT=vsl(sqd, ts * 4 + b),
                          rhs=wsq[:, di * 80:di * 80 + 60],
                          start=True, stop=True)
            snr = WK.tile([60, 240], F32, tag="snr", name=f"snr_{dname}_{ts}")
            v.reciprocal(out=snr[:], in_=sps[:])
            RNs = WK.tile([60, 240], F32, tag="RNs", name=f"RNs_{dname}_{ts}")
            sc.activation(out=RNs[:], in_=snr[:], func=AF.Sqrt)
            for b in range(BL):
                # wu_cat (100, 60) = wsq[:, dir singles] * u columns
                wu = WK.tile([100, 60], F32, tag="wu", name=f"wu_{dname}_{ts}_{b}")
                v.tensor_tensor(out=wu[:], in0=wsq[:, di * 80:di * 80 + 60],
                                in1=apv(U12[:], 0, b * 3, [(1, 3), (0, L)]), op=OP.mult)
                nums = MP.tile([60, 60], F32, tag="pA", bufs=2, name=f"nums_{dname}_{ts}_{b}")
                te.matmul(out=nums[:], lhsT=vsl(seqd, ts * 4 + b), rhs=wu[:],
                          start=True, stop=True)
                RNc = WK.tile([60, 60], F32, tag="RNc", name=f"RNc_{dname}_{ts}_{b}")
                v.tensor_tensor(out=RNc[:], in0=apv(RNs[:], 0, b * 60, [(1, 60)], pcnt=60),
                                in1=rbA[:, b * 60:(b + 1) * 60], op=OP.mult)
                # mv writes: full -> cols di*80+[0:20]; am|amax -> di*80+[40:80]
                v.tensor_tensor(out=mvt[b][:, di * 80:di * 80 + L],
                                in0=nums[:, 0:L], in1=RNc[:, 0:L], op=OP.mult)
                v.tensor_tensor(out=mvt[b][:, di * 80 + 2 * L:di * 80 + 4 * L],
                                in0=nums[:, L:3 * L], in1=RNc[:, L:3 * L], op=OP.mult)

        # ---- pairwise max ----
        # cos(w_l*v1_i, w_l*v2_j): mats = (w_l^2*v1).T @ (v2*rn2[j,l]); post-scale rn1[i,l].
        wmx = wsq[:, di * 80 + 60: di * 80 + 80]
        # per-side pairwise norms RNp[side] (60, 80) [pos, b*20+l]
        RNp = {}
        for side in (0, 1):
            pps = MP.tile([60, 80], F32, tag="pA", bufs=2, name=f"pps_{dname}_{side}")
            for b in range(BL):
                te.matmul(out=pps[:, b * L:(b + 1) * L],
                          lhsT=vsl(sqd, side * 4 + b), rhs=wmx,
                          start=True, stop=True)
            prr = WK.tile([60, 80], F32, tag="prr", name=f"prr_{dname}_{side}")
            v.reciprocal(out=prr[:], in_=pps[:])
            rp = CP.tile([60, 80], F32, name=f"RNp_{dname}_{side}")
            sc.activation(out=rp[:], in_=prr[:], func=AF.Sqrt)
            RNp[side] = rp
        # partition-broadcast of RNp over h via sb->sb DMA: (100, (b,l,j))
        rnb = {}
        for side in (0, 1):
            pst2 = MP.tile([80, 60], F32, tag="pB", bufs=1, name=f"rnpt_{dname}_{side}")
            te.transpose(out=pst2[:], in_=RNp[side][:], identity=ident[:60, :60])
            rnpt = WK.tile([80, 60], F32, tag="rnpt", name=f"rnptS_{dname}_{side}")
            v.tensor_copy(out=rnpt[:], in_=pst2[:])
            flat = WK.tile([1, 4800], F32, tag="rnflat", bufs=1, name=f"rnflat_{dname}_{side}")
            rsT = rnpt[:].ap[0][0]
            src = AP(rnpt[:].tensor, rnpt[:].offset, [[rsT, 80], [1, 60]])
            sy.dma_start(out=flat[:], in_=src)
            t0 = VWP.tile([100, 4800], F32, tag="rnb", name=f"rnb_{dname}_{side}")
            gp.partition_broadcast(t0[:], flat[:1, :])
            rnb[side] = t0
        for ts in (0, 1):
            us = 1 - ts
            mvt = mvp if ts == 0 else mvh
            vw = VWP.tile([100, 4800], F32, tag="vw", bufs=1, name=f"vw_{dname}_{ts}")
            v.tensor_tensor(
                out=vw[:],
                in0=apv(seqd[:], 0, ts * 4, [(1, 4), (0, L), (8, 60)]),
                in1=apv(wmx, 0, 0, [(0, 4), (1, L), (0, 60)]),
                op=OP.mult)
            v2n = VWP.tile([100, 4800], F32, tag="v2n", bufs=1, name=f"v2n_{dname}_{ts}")
            v.tensor_tensor(
                out=v2n[:],
                in0=apv(seqd[:], 0, us * 4, [(1, 4), (0, L), (8, 60)]),
                in1=rnb[us][:], op=OP.mult)
            for b in range(BL):
                grp = MPG.tile([60, 1280], F32, tag="grp", name=f"grp_{dname}_{ts}_{b}")
                for l in range(L):
                    te.matmul(out=grp[:, l * 64:l * 64 + 60],
                              lhsT=vw[:, (b * L + l) * 60:(b * L + l + 1) * 60],
                              rhs=v2n[:, (b * L + l) * 60:(b * L + l + 1) * 60],
                              start=True, stop=True)
                stg = WK.tile([60, L], F32, tag="stg", name=f"stg_{dname}_{ts}_{b}")
                v.reduce_max(out=stg[:], in_=apv(grp[:], 0, 0, [(64, L), (1, 60)]),
                             axis=AX.X)
                v.tensor_tensor(out=mvt[b][:, di * 80 + L:di * 80 + 2 * L],
                                in0=stg[:], in1=RNp[ts][:, b * L:(b + 1) * L],
                                op=OP.mult)

    m_es.close()

    # ================= mv -> transposed agg inputs =================
    ap_es = _ES()
    AP_ = ap_es.enter_context(tc.tile_pool(name="aggprep_ps", bufs=2, space="PSUM"))
    mvagg = [CP.tile([80, NTOK], F32, name="mvagg_0"),
             CP.tile([81, NTOK], F32, name="mvagg_1")]
    sy.dma_start(out=mvagg[1][80:81, :], in_=ones_row[:])
    for side in (0, 1):
        mvt = mvp if side == 0 else mvh
        for k in range(2):
            ps = AP_.tile([80, 240], F32, tag="mvtp", name=f"mvtp_{side}_{k}")
            for b in range(BL):
                te.transpose(out=ps[:, b * 60:(b + 1) * 60],
                             in_=mvt[b][:, k * 80:(k + 1) * 80],
                             identity=ident[:60, :60])
            dst = apv(mvagg[k][:], 0, side * 4, [(1, 4), (8, 60)], pcnt=80)
            src = apv(ps[:], 0, 0, [(60, 4), (1, 60)])
            v.tensor_copy(out=dst, in_=src)
    mvaggr = [CP.tile([80, NTOK], F32, name="mvaggr_0"),
              CP.tile([81, NTOK], F32, name="mvaggr_1")]
    for k in range(2):
        pc = 81 if k == 1 else 80
        v.tensor_copy(out=apv(mvaggr[k][:], 0, 0, [(8, 60), (1, 8)], pcnt=pc),
                      in_=revt(mvagg[k][:], 0, pc, 0, [(8, 60), (1, 8)]))
    ap_es.close()

    # ================= aggregation scan =================
    seqa = {ws: CP.tile([H, NTOK], F32, name=f"seqa_{ws}") for ws in ("f", "b")}
    run_scan("agg", agg_wihT, agg_whhT, {"f": mvagg, "b": mvaggr}, [80, 80], seqa)

    # ================= FC head =================
    fp_es = _ES()
    FP = fp_es.enter_context(tc.tile_pool(name="fc_ps", bufs=1, space="PSUM"))
    xk = [seqa["f"][:, 472:476], seqa["b"][:, 472:476],
          seqa["f"][:, 476:480], seqa["b"][:, 476:480]]
    x1 = []
    for m in range(2):
        ps = FP.tile([100, 4], F32, tag="fc1ps", name=f"fc1ps_{m}")
        for k in range(4):
            te.matmul(out=ps[:], lhsT=fc1T[k][:, m * 100:(m + 1) * 100], rhs=xk[k],
                      start=(k == 0), stop=(k == 3))
        t0 = CP.tile([100, 4], F32, name=f"x1_{m}")
        sc.activation(out=t0[:], in_=ps[:], func=AF.Tanh, bias=fc1b[:, m:m + 1])
        x1.append(t0)
    ps2 = FP.tile([CLS, 4], F32, tag="fc2ps", name="fc2ps")
    for k in range(2):
        te.matmul(out=ps2[:], lhsT=fc2T[k][:], rhs=x1[k][:],
                  start=(k == 0), stop=(k == 1))
    osb = CP.tile([CLS, 4], F32, name="osb")
    sc.activation(out=osb[:], in_=ps2[:], func=AF.Identity, bias=fc2b[:, 0:1])
    sy.dma_start(out=io["out"].rearrange("b c -> c b"), in_=osb[:])
    fp_es.close()

    es.close()


_CACHE = {}


def build():
    if "nc" in _CACHE:
        return _CACHE["nc"]
    nc = bacc.Bacc("TRN2", target_bir_lowering=False, debug=False, num_devices=NCORE)
    io = {}
    io["p"] = nc.dram_tensor("p", [BL, S], I32, kind="ExternalInput").ap()
    io["h"] = nc.dram_tensor("h", [BL, S], I32, kind="ExternalInput").ap()
    shapes = {
        "word_emb": [V, D],
        "fc1_W": [2 * H, DH], "fc1_b": [2 * H], "fc2_W": [CLS, 2 * H], "fc2_b": [CLS],
    }
    for pre, din in (("ctx", D), ("agg", AGG_IN)):
        for d in ("f", "b"):
            shapes[f"{pre}_Wih_{d}"] = [DH, din]
            shapes[f"{pre}_Whh_{d}"] = [DH, H]
            shapes[f"{pre}_bih_{d}"] = [DH]
            shapes[f"{pre}_bhh_{d}"] = [DH]
    for i in range(1, 9):
        shapes[f"w{i}"] = [H, L]
    for name in PARAM_NAMES:
        io[name] = nc.dram_tensor(name, shapes[name], F32, kind="ExternalInput").ap()
    io["out"] = nc.dram_tensor("out", [BL, CLS], F32, kind="ExternalOutput").ap()

    with tile.TileContext(nc) as tc:
        emit(nc, tc, io)
    nc.compile()
    _CACHE["nc"] = nc
    return nc


def kernel(**inputs):
    nc = build()
    base = {k: np.ascontiguousarray(np.asarray(inputs[k], dtype=np.float32))
            for k in PARAM_NAMES}
    in_maps = []
    for c in range(NCORE):
        m = dict(base)
        m["p"] = np.ascontiguousarray(np.asarray(inputs["p"], dtype=np.int32)[c * BL:(c + 1) * BL])
        m["h"] = np.ascontiguousarray(np.asarray(inputs["h"], dtype=np.int32)[c * BL:(c + 1) * BL])
        in_maps.append(m)
    res = run_bass_kernel_spmd(nc, in_maps, list(range(NCORE)))
    out = np.concatenate([res.results[c]["out"] for c in range(NCORE)], axis=0)
    return out.astype(np.float32)
